# revision 37
# baseline (speedup 1.0000x reference)
"""Causal chunked prefill (multi-head attention block) on 8 Trainium2 cores.

Full inputs in, full output out.  Sharding: 8 cores = batch(2) x head-group(4).
Each core computes q/k/v projections for its 4 heads (256 channels), causal
softmax attention, and a partial output projection (its 256 ctx channels
through the matching 256 rows of Wo^T).  Host sums the 4 partials per batch
element and adds bo (+ Wo @ bv, folded in since softmax rows sum to 1).

Design (all matmul operands bf16; f32 PSUM accumulation):
  - Host pre-casts x/W to bf16 -> DMA lands directly in resident SBUF tiles.
  - Scores for BOTH heads of a pair land in one [128,1024] PSUM tile
    (2 banks); ONE exp activation per (g, k-block) covers both heads
    (strided AP restricted to causally valid columns).
  - ctx computed directly as [q, dv]: lhsT = A^T block [128k,128q],
    rhs = [V | 1] [128k,65] -> psum [128q,65]; col 64 = softmax denominator.
    All 8 (h2,qq) accumulator regions pack into 2 PSUM banks: the bank is
    memset once per stream and every matmul uses start=False (a start=True
    would clobber the whole bank's other in-flight regions).
  - Software pipelining: ctx matmuls trail the score/exp stream by two
    k-blocks, and the NEXT strip's projection matmuls are split into
    2-matmul chunks drip-fed into the attention stream (sharing the
    out-projection's PSUM slots), so the PE never starves while the
    scalar engine runs exp.  Writers must be emitted before their readers
    (the tile framework only orders deps backward in emission order), so
    strip 0's V projection is emitted ahead of its own attention.
  - Batched reciprocal over the 4 denominators (stride-65 AP); normalize
    split DVE/ACT; PE transpose (bf16 identity) feeds the output
    projection; partial outputs stored bf16 (host sums in f32).
  - Engine budget: PE matmuls; ACT exp + half the normalizes + ctxT
    evictions; DVE projections evictions/recip/norm/out; Pool(gpsimd)
    causal trimask multiplies (SBUF-only ops).
"""

import sys

import numpy as np
import ml_dtypes

sys.path.insert(0, "/opt/trn_rl_repo")

import concourse.bass as bass
import concourse.bacc as bacc
import concourse.mybir as mybir
import concourse.tile as tile
from concourse.bass_utils import run_bass_kernel_spmd

F32 = mybir.dt.float32
BF16 = mybir.dt.bfloat16
AF = mybir.ActivationFunctionType
ALU = mybir.AluOpType
NPBF16 = ml_dtypes.bfloat16

B, S, D = 2, 2048, 1024
H, HD = 16, 64
NCORES = 8
HGROUPS = 4          # head groups (cores per batch element)
HPC = H // HGROUPS   # heads per core = 4
C = HPC * HD         # channels per core = 256
ET = D // 128        # e (contraction) tiles = 8
NSTRIP = S // 512    # 512-wide query strips = 4


def _rr(ap, *args, **kw):
    return ap.rearrange(*args, **kw)


def build_program():
    nc = bacc.Bacc(None)

    xT = nc.dram_tensor("xT", [D, S], BF16, kind="ExternalInput")
    wqT = nc.dram_tensor("wqT", [D, C], BF16, kind="ExternalInput")
    wkT = nc.dram_tensor("wkT", [D, C], BF16, kind="ExternalInput")
    wvT = nc.dram_tensor("wvT", [D, C], BF16, kind="ExternalInput")
    woT = nc.dram_tensor("woT", [C, D], BF16, kind="ExternalInput")
    bq = nc.dram_tensor("bq", [2, 128, 1], F32, kind="ExternalInput")  # /8 on host
    bk = nc.dram_tensor("bk", [2, 128, 1], F32, kind="ExternalInput")
    out = nc.dram_tensor("out", [S, D], BF16, kind="ExternalOutput")

    with tile.TileContext(nc) as tc:
        _emit(nc, tc, xT, wqT, wkT, wvT, woT, bq, bk, out)
    nc.finalize()
    return nc


def _emit(nc, tc, xT, wqT, wkT, wvT, woT, bq, bk, out):
    with (
        tc.tile_pool(name="const", bufs=1) as constp,
        tc.tile_pool(name="xp", bufs=1) as xp,
        tc.tile_pool(name="wp", bufs=1) as wp,
        tc.tile_pool(name="actp", bufs=1) as actp,
        tc.tile_pool(name="apool", bufs=6) as apool,
        tc.tile_pool(name="ctxp", bufs=8) as ctxp,
        tc.tile_pool(name="ctp", bufs=4) as ctp,
        tc.tile_pool(name="rcp", bufs=8) as rcp,
        tc.tile_pool(name="outp", bufs=4) as outp,
        tc.tile_pool(name="psm", bufs=2, space="PSUM") as psm,
        tc.tile_pool(name="pctx", bufs=1, space="PSUM") as pctx,
        tc.tile_pool(name="po", bufs=2, space="PSUM") as po,
    ):
        # ---- constants -------------------------------------------------
        trimask2 = constp.tile([128, 256], BF16)  # two copies: 1 where col>=row
        nc.vector.memset(trimask2[:], 1.0)
        for t in range(2):
            nc.gpsimd.affine_select(
                out=trimask2[:, t * 128 : (t + 1) * 128],
                in_=trimask2[:, t * 128 : (t + 1) * 128],
                compare_op=ALU.is_ge,
                fill=0.0, base=0, pattern=[[1, 128]], channel_multiplier=-1,
            )
        ident = constp.tile([128, 128], BF16)     # identity for PE transpose
        nc.vector.memset(ident[:], 1.0)
        nc.gpsimd.affine_select(
            out=ident[:], in_=ident[:],
            compare_op=ALU.is_equal,
            fill=0.0, base=0, pattern=[[1, 128]], channel_multiplier=-1,
        )
        bq_sb = constp.tile([128, 2], F32)
        bk_sb = constp.tile([128, 2], F32)
        for g in range(2):
            nc.sync.dma_start(out=bq_sb[:, g : g + 1], in_=bq[g])
            nc.sync.dma_start(out=bk_sb[:, g : g + 1], in_=bk[g])

        # ---- resident weights & x (DMA directly, already bf16) ---------
        wq_sb = wp.tile([128, ET * C], BF16, tag="wq")
        wk_sb = wp.tile([128, ET * C], BF16, tag="wk")
        nc.sync.dma_start(
            out=_rr(wq_sb[:], "p (e c) -> p e c", c=C),
            in_=_rr(wqT[:], "(e p) c -> p e c", p=128),
        )
        xt = [xp.tile([128, S], BF16, tag=f"xt{e}", name=f"xt{e}") for e in range(ET)]
        for et in range(ET):
            nc.sync.dma_start(
                out=xt[et][:, 0:512], in_=xT[et * 128 : (et + 1) * 128, 0:512])
        nc.sync.dma_start(
            out=_rr(wk_sb[:], "p (e c) -> p e c", c=C),
            in_=_rr(wkT[:], "(e p) c -> p e c", p=128),
        )
        wv_sb = wp.tile([128, ET * C], BF16, tag="wv")
        nc.sync.dma_start(
            out=_rr(wv_sb[:], "p (e c) -> p e c", c=C),
            in_=_rr(wvT[:], "(e p) c -> p e c", p=128),
        )
        wo_sb = [wp.tile([128, D], BF16, tag=f"wo{t}", name=f"wo{t}") for t in range(2)]
        for t in range(2):
            nc.sync.dma_start(out=wo_sb[t][:], in_=woT[t * 128 : (t + 1) * 128, :])
        for ic4 in range(1, NSTRIP):
            for et in range(ET):
                nc.sync.dma_start(
                    out=xt[et][:, ic4 * 512 : (ic4 + 1) * 512],
                    in_=xT[et * 128 : (et + 1) * 128, ic4 * 512 : (ic4 + 1) * 512])

        # ---- activations -----------------------------------------------
        qt = [actp.tile([128, S], BF16, tag=f"qt{g}", name=f"qt{g}") for g in range(2)]
        kt = [actp.tile([128, S], BF16, tag=f"kt{g}", name=f"kt{g}") for g in range(2)]
        vone = actp.tile([128, 4 * NSTRIP * HPC * 65], BF16, tag="vone")
        nc.vector.memset(vone[:], 1.0)

        def proj_chunks(ic4):
            """Projection work for strip ic4, split into small closures (2
            matmuls each) that get interleaved into the attention stream to
            fill PE idle time while the scalar engine runs exp.  Each psum
            group borrows a slot from the out-projection pool (time-disjoint
            usage keeps total PSUM banks at 8)."""
            sl = slice(ic4 * 512, (ic4 + 1) * 512)
            chunks = []

            def qk_group(w_sb, b_sb, dst, scale, g):
                state = {}
                def chunk(eti):
                    if eti == 0:
                        state["ps"] = po.tile([128, 512], F32, tag="po", name="pp")
                    ps = state["ps"]
                    for et in (2 * eti, 2 * eti + 1):
                        nc.tensor.matmul(
                            ps[:],
                            lhsT=w_sb[:, et * C + g * 128 : et * C + g * 128 + 128],
                            rhs=xt[et][:, sl],
                            start=(et == 0), stop=(et == ET - 1),
                        )
                    if eti == 3:
                        # dst = ps*scale + bias (bias pre-scaled on host for q)
                        nc.vector.tensor_scalar(
                            dst[g][:, sl], ps[:], scale, b_sb[:, g : g + 1],
                            op0=ALU.mult, op1=ALU.add,
                        )
                return chunk

            def v_group(jb):
                state = {}
                def chunk(eti):
                    if eti == 0:
                        state["ps"] = po.tile([128, 512], F32, tag="po", name="pv")
                    ps = state["ps"]
                    for et in (2 * eti, 2 * eti + 1):
                        nc.tensor.matmul(
                            ps[:, 0:C],
                            lhsT=xt[et][:, jb * 128 : (jb + 1) * 128],
                            rhs=wv_sb[:, et * C : (et + 1) * C],
                            start=(et == 0), stop=(et == ET - 1),
                        )
                    if eti == 3:
                        dstv = _rr(vone[:, jb * HPC * 65 : (jb + 1) * HPC * 65],
                                   "p (h c) -> p h c", c=65)
                        nc.vector.tensor_copy(
                            dstv[:, :, 0:64],
                            _rr(ps[:, 0:C], "p (h c) -> p h c", c=HD))
                return chunk

            def expand(groups):
                out = []
                for c in groups:
                    out += [(lambda c=c, i=i: c(i)) for i in range(4)]
                return out

            qk_gs = [qk_group(w_sb, b_sb, dst, scale, g)
                     for w_sb, b_sb, dst, scale in
                     ((wq_sb, bq_sb, qt, 0.125), (wk_sb, bk_sb, kt, 1.0))
                     for g in range(2)]
            v_gs = [v_group(jb) for jb in range(4 * ic4, 4 * ic4 + 4)]
            if ic4 == 0:
                # strip 0's V MUST be emitted before its own attention stream
                # (a reader emitted before its writer sees stale data); only
                # the g=1 q/k projections can drip into the g=0 stream.
                head = expand([qk_gs[0], qk_gs[2]]) + expand(v_gs)
                rest = expand([qk_gs[1], qk_gs[3]])
            else:
                head = []
                rest = expand(qk_gs) + expand(v_gs)
            return head, rest

        # ---- main loop: strips of 512 queries ---------------------------
        # strip 0: emit the g=0 q/k projections up front so attention (and
        # the exp stream) starts ASAP; everything else drip-feeds.
        head0, rest0 = proj_chunks(0)
        for ch in head0:
            ch()
        pend0 = rest0
        for qp in range(NSTRIP):
            ctx_sb = [ctxp.tile([128, C], BF16, tag="ctx", name=f"ctx{qq}")
                      for qq in range(4)]
            # projection chunks of the next strip, drip-fed into this strip's
            # attention stream at one point per (g, jb) iteration
            pending = list(pend0)
            pend0 = []
            if qp + 1 < NSTRIP:
                h1, r1 = proj_chunks(qp + 1)
                pending += h1 + r1
            # +4: leave a few chunks to pad the fin phase's PE bubbles
            points = 2 * (4 * qp + 4) + 4
            for g in range(2):
                # 8 accumulator regions (h2 x qq) in 2 banks; start=False
                # accumulation onto a memset bank (start=True would clobber
                # the bank's other regions)
                pc = [pctx.tile([128, 260], F32, tag=f"pch{h2}", name=f"pc{h2}")
                      for h2 in range(2)]
                for h2 in range(2):
                    nc.vector.memset(pc[h2][:], 0.0)

                def emit_ctx(jb, r, a_sb):
                    for h2 in range(2):
                        h = 2 * g + h2
                        vs = vone[:, jb * HPC * 65 + h * 65
                                  : jb * HPC * 65 + (h + 1) * 65]
                        for qq in range(max(r, 0), 4):
                            nc.tensor.matmul(
                                pc[h2][:, qq * 65 : qq * 65 + 65],
                                lhsT=a_sb[:, h2 * 512 + qq * 128
                                          : h2 * 512 + (qq + 1) * 128],
                                rhs=vs,
                                start=False, stop=(r == qq),
                            )

                def inject():
                    nonlocal points
                    n = -(-len(pending) // points) if pending else 0
                    points -= 1
                    for _ in range(min(n, len(pending))):
                        pending.pop(0)()

                carries = []  # ctx matmuls trail scores by two blocks so the
                # PE has other work while ACT computes this block's exp
                for jb in range(4 * qp + 4):
                    r = jb - 4 * qp
                    c0 = max(r, 0) * 128
                    sp = psm.tile([128, 1024], F32, tag="s", name="sp")
                    for h2 in range(2):
                        nc.tensor.matmul(
                            sp[:, h2 * 512 + c0 : h2 * 512 + 512],
                            lhsT=kt[g][h2 * 64 : h2 * 64 + 64,
                                       jb * 128 : (jb + 1) * 128],
                            rhs=qt[g][h2 * 64 : h2 * 64 + 64,
                                      qp * 512 + c0 : (qp + 1) * 512],
                            start=True, stop=True,
                        )
                    a_sb = apool.tile([128, 1024], BF16, tag="a")
                    if c0:
                        spv = _rr(sp[:], "p (h w) -> p h w", w=512)[:, :, c0:512]
                        av = _rr(a_sb[:], "p (h w) -> p h w", w=512)[:, :, c0:512]
                    else:
                        spv, av = sp[:], a_sb[:]
                    nc.scalar.activation(av, spv, AF.Exp)
                    if r >= 0:
                        mv = _rr(a_sb[:], "p (h w) -> p h w", w=512)[
                            :, :, c0 : c0 + 128]
                        nc.gpsimd.tensor_tensor(
                            mv, mv, _rr(trimask2[:], "p (h w) -> p h w", w=128),
                            op=ALU.mult)
                    carries.append((jb, r, a_sb))
                    if len(carries) > 2:
                        emit_ctx(*carries.pop(0))
                        inject()
                for cr in carries:
                    emit_ctx(*cr)
                    inject()
                for h2 in range(2):
                    h = 2 * g + h2
                    rc = rcp.tile([128, 4], F32, tag="rc")
                    nc.vector.reciprocal(
                        rc[:], _rr(pc[h2][:], "p (q c) -> p q c", c=65)[:, :, 64:65])
                    for qq in range(4):
                        # split the normalize convoy across DVE and ACT
                        if h2 == 0:
                            nc.vector.tensor_scalar(
                                ctx_sb[qq][:, h * 64 : (h + 1) * 64],
                                pc[h2][:, qq * 65 : qq * 65 + 64],
                                rc[:, qq : qq + 1], None,
                                op0=ALU.mult,
                            )
                        else:
                            nc.scalar.activation(
                                ctx_sb[qq][:, h * 64 : (h + 1) * 64],
                                pc[h2][:, qq * 65 : qq * 65 + 64],
                                AF.Copy, scale=rc[:, qq : qq + 1],
                            )
            # ---- transpose ctx + output projection per 128-row block ---
            for qq in range(4):
                ib = 4 * qp + qq
                ctxTb = ctp.tile([128, C], BF16, tag="ctxT")
                for t in range(2):
                    pt = psm.tile([128, 128], BF16, tag="s", name="pt")
                    nc.tensor.transpose(
                        pt[:], ctx_sb[qq][:, t * 128 : (t + 1) * 128], ident[:])
                    nc.scalar.activation(
                        ctxTb[:, t * 128 : (t + 1) * 128], pt[:], AF.Copy)
                if pending:
                    pending.pop(0)()
                for ec in range(2):
                    pob = po.tile([128, 512], F32, tag="po", name="pob")
                    for t in range(2):
                        nc.tensor.matmul(
                            pob[:],
                            lhsT=ctxTb[:, t * 128 : (t + 1) * 128],
                            rhs=wo_sb[t][:, ec * 512 : (ec + 1) * 512],
                            start=(t == 0), stop=(t == 1),
                        )
                    o_sb = outp.tile([128, 512], BF16, tag="ob")
                    nc.vector.tensor_copy(o_sb[:], pob[:])
                    nc.sync.dma_start(
                        out=out[ib * 128 : (ib + 1) * 128,
                                ec * 512 : (ec + 1) * 512],
                        in_=o_sb[:],
                    )
            for ch in pending:
                ch()


_NC = None


def _get_program():
    global _NC
    if _NC is None:
        _NC = build_program()
    return _NC


def make_in_maps(x, Wq, bq, Wk, bk, Wv, Wo):
    x = np.asarray(x, np.float32)
    in_maps = []
    for c in range(NCORES):
        b, hg = divmod(c, HGROUPS)
        sl = slice(hg * C, (hg + 1) * C)
        in_maps.append({
            "xT": np.ascontiguousarray(x[b].T).astype(NPBF16),
            "wqT": np.ascontiguousarray(np.asarray(Wq, np.float32)[sl, :].T).astype(NPBF16),
            "wkT": np.ascontiguousarray(np.asarray(Wk, np.float32)[sl, :].T).astype(NPBF16),
            "wvT": np.ascontiguousarray(np.asarray(Wv, np.float32)[sl, :].T).astype(NPBF16),
            "woT": np.ascontiguousarray(np.asarray(Wo, np.float32)[:, sl].T).astype(NPBF16),
            "bq": (np.asarray(bq, np.float32)[sl] * 0.125).reshape(2, 128, 1).copy(),
            "bk": np.asarray(bk, np.float32)[sl].reshape(2, 128, 1).copy(),
        })
    return in_maps


def gather(results, bv, Wo, bo):
    outf = np.zeros((B, S, D), np.float32)
    for c in range(NCORES):
        outf[c // HGROUPS] += np.asarray(results[c]["out"], np.float32)
    # softmax rows sum to 1, so the v-bias contributes Wo @ bv to every row
    bo_eff = (np.asarray(bo, np.float64)
              + np.asarray(Wo, np.float64) @ np.asarray(bv, np.float64))
    outf += bo_eff.astype(np.float32)[None, None, :]
    return outf


def run_sharded(inputs, trace=False, **kw):
    nc = _get_program()
    in_maps = make_in_maps(
        inputs["x"], inputs["Wq"], inputs["bq"], inputs["Wk"], inputs["bk"],
        inputs["Wv"], inputs["Wo"])
    bkr = run_bass_kernel_spmd(nc, in_maps, list(range(NCORES)), trace=trace, **kw)
    return gather(bkr.results, inputs["bv"], inputs["Wo"], inputs["bo"]), bkr


def kernel(x, Wq, bq, Wk, bk, Wv, bv, Wo, bo):
    out, _ = run_sharded(dict(x=x, Wq=Wq, bq=bq, Wk=Wk, bk=bk, Wv=Wv, bv=bv,
                              Wo=Wo, bo=bo))
    return out
